# revision 1
# baseline (speedup 1.0000x reference)
"""Bass/Trainium2 kernel for nn_CrossAttentionLayer.

out = softmax((x_q Wq^T + bq)(x_k Wk^T + bk)^T) (x_v Wv^T + bv)

Sharding: data-parallel over batch B=8 across the 8 NeuronCores.
Exact math simplifications used:
  - bk drops out of softmax (adds a per-row constant to the logits).
  - bv is added on the host (softmax rows sum to 1, so attn @ (v0 + bv)
    = attn @ v0 + bv).
  - softmax normalization (divide by row-sum) commutes with the PV
    matmul, so the device returns the unnormalized PV product plus
    row-sums and the host divides.
Device layout: scores are computed TRANSPOSED ([key, query] tiles) so
the PV matmul can consume v in its natural [key, d] layout with no
on-chip transpose of the attention matrix; row-sums over the key
(partition) axis are computed with a ones-vector matmul on the PE.
"""

import sys

if "/opt/trn_rl_repo" not in sys.path:
    sys.path.insert(0, "/opt/trn_rl_repo")

import numpy as np

B = 8          # batch == number of cores
D = 1024       # model/latent dim
N = 2048       # tokens (queries == keys)
P = 128        # partitions
DC = D // P    # 8 chunks of the d/e axis
JT = N // P    # 16 key tiles
F = 512        # matmul moving free dim (fp32 max)
NB = N // F    # 4 query blocks

_CACHE = {}


def _build_nc():
    import concourse.bass as bass
    import concourse.mybir as mybir
    import concourse.tile as tile
    from concourse import bacc
    from concourse.masks import make_identity
    from contextlib import ExitStack

    f32 = mybir.dt.float32
    f32r = mybir.dt.float32r
    EXP = mybir.ActivationFunctionType.Exp

    nc = bacc.Bacc("TRN2", target_bir_lowering=False, debug=False, num_devices=B)

    xqt = nc.dram_tensor("xqt", [D, N], f32r, kind="ExternalInput").ap()
    xkt = nc.dram_tensor("xkt", [D, N], f32r, kind="ExternalInput").ap()
    xvt = nc.dram_tensor("xvt", [D, N], f32r, kind="ExternalInput").ap()
    wqt = nc.dram_tensor("wqt", [D, D], f32r, kind="ExternalInput").ap()
    wkt = nc.dram_tensor("wkt", [D, D], f32r, kind="ExternalInput").ap()
    wvt = nc.dram_tensor("wvt", [D, D], f32r, kind="ExternalInput").ap()
    bqt = nc.dram_tensor("bqt", [P, DC], f32, kind="ExternalInput").ap()

    v_int = nc.dram_tensor("v_int", [N, D], f32r).ap()
    qt_int = nc.dram_tensor("qt_int", [D, N], f32r).ap()

    acct = nc.dram_tensor("acct", [D, N], f32, kind="ExternalOutput").ap()
    rowsum = nc.dram_tensor("rowsum", [NB, F], f32, kind="ExternalOutput").ap()

    with ExitStack() as ctx:
        tc = ctx.enter_context(tile.TileContext(nc))
        big = ctx.enter_context(tc.tile_pool(name="big", bufs=2))
        ktp = ctx.enter_context(tc.tile_pool(name="ktp", bufs=1))
        xjp = ctx.enter_context(tc.tile_pool(name="xjp", bufs=3))
        natp = ctx.enter_context(tc.tile_pool(name="natp", bufs=1))
        stp = ctx.enter_context(tc.tile_pool(name="stp", bufs=2))
        qsp = ctx.enter_context(tc.tile_pool(name="qsp", bufs=2))
        vip = ctx.enter_context(tc.tile_pool(name="vip", bufs=2))
        cst = ctx.enter_context(tc.tile_pool(name="cst", bufs=1))
        psa = ctx.enter_context(tc.tile_pool(name="psa", bufs=3, space="PSUM"))
        pst = ctx.enter_context(tc.tile_pool(name="pst", bufs=2, space="PSUM"))
        psm = ctx.enter_context(tc.tile_pool(name="psm", bufs=2, space="PSUM"))
        psr = ctx.enter_context(tc.tile_pool(name="psr", bufs=1, space="PSUM"))

        ident = cst.tile([P, P], f32, name="ident", tag="c_id")
        make_identity(nc, ident)
        ones_f = cst.tile([P, 1], f32, name="ones_f", tag="c_onesf")
        nc.vector.memset(ones_f, 1.0)
        ones = cst.tile([P, 1], f32r, name="ones", tag="c_ones")
        nc.vector.tensor_copy(ones, ones_f)
        bq_sb = cst.tile([P, DC], f32, name="bq_sb", tag="c_bq")

        kt_sb = ktp.tile([P, DC, N], f32r, name="kt_sb", tag="kt")

        def load_w(dst, src):
            srcr = src.rearrange("(c p) e -> p c e", p=P)
            for c in range(DC):
                nc.sync.dma_start(out=dst[:, c, :], in_=srcr[:, c, :])

        xkr = xkt.rearrange("(c p) n -> p c n", p=P)
        xt0 = xjp.tile([P, DC, P], f32r, name="xt_k", tag="xj")
        nc.sync.dma_start(out=xt0, in_=xkr[:, :, 0:P])
        wk_sb = big.tile([P, DC, D], f32r, name="wk_sb", tag="big")
        load_w(wk_sb, wkt)
        wv_sb = big.tile([P, DC, D], f32r, name="wv_sb", tag="big")

        # ---- Phase 1a: kT[e, j] = (x_k Wk^T)^T, resident in SBUF ----
        for jt in range(JT):
            if jt == 0:
                xt = xt0
            else:
                xt = xjp.tile([P, DC, P], f32r, name="xt_k", tag="xj")
                nc.sync.dma_start(out=xt, in_=xkr[:, :, jt * P:(jt + 1) * P])
            knat = natp.tile([P, D], f32, name="knat", tag="nat")
            for nd in range(2):
                ps = psa.tile([P, F], f32, name="ps_k", tag="psa")
                for dp in range(DC):
                    nc.tensor.matmul(
                        ps,
                        lhsT=xt[:, dp, :],
                        rhs=wk_sb[:, dp, nd * F:(nd + 1) * F],
                        start=(dp == 0),
                        stop=(dp == DC - 1),
                    )
                nc.vector.tensor_copy(knat[:, nd * F:(nd + 1) * F], ps)
            for e in range(DC):
                pt = pst.tile([P, P], f32, name="pt_k", tag="pst")
                nc.tensor.transpose(pt, knat[:, e * P:(e + 1) * P], ident)
                nc.scalar.copy(kt_sb[:, e, jt * P:(jt + 1) * P], pt)

        # ---- Phase 1b: v = x_v Wv^T (no bias) -> DRAM ----
        load_w(wv_sb, wvt)
        xvr = xvt.rearrange("(c p) n -> p c n", p=P)
        for jt in range(JT):
            xt = xjp.tile([P, DC, P], f32r, name="xt_v", tag="xj")
            nc.sync.dma_start(out=xt, in_=xvr[:, :, jt * P:(jt + 1) * P])
            for nd in range(2):
                ps = psa.tile([P, F], f32, name="ps_v", tag="psa")
                for dp in range(DC):
                    nc.tensor.matmul(
                        ps,
                        lhsT=xt[:, dp, :],
                        rhs=wv_sb[:, dp, nd * F:(nd + 1) * F],
                        start=(dp == 0),
                        stop=(dp == DC - 1),
                    )
                st = stp.tile([P, F], f32r, name="st_v", tag="st")
                nc.vector.tensor_copy(st, ps)
                nc.sync.dma_start(
                    out=v_int[jt * P:(jt + 1) * P, nd * F:(nd + 1) * F], in_=st
                )

        # ---- Phase 1c: qT[e, i] = (x_q Wq^T + bq)^T -> DRAM ----
        wq_sb = big.tile([P, DC, D], f32r, name="wq_sb", tag="big")
        load_w(wq_sb, wqt)
        nc.sync.dma_start(out=bq_sb, in_=bqt)
        xqr = xqt.rearrange("(c p) n -> p c n", p=P)
        qtr = qt_int.rearrange("(c p) n -> p c n", p=P)
        for it in range(JT):
            xt = xjp.tile([P, DC, P], f32r, name="xt_q", tag="xj")
            nc.sync.dma_start(out=xt, in_=xqr[:, :, it * P:(it + 1) * P])
            qnat = natp.tile([P, D], f32, name="qnat", tag="nat")
            for nd in range(2):
                ps = psa.tile([P, F], f32, name="ps_q", tag="psa")
                for dp in range(DC):
                    nc.tensor.matmul(
                        ps,
                        lhsT=xt[:, dp, :],
                        rhs=wq_sb[:, dp, nd * F:(nd + 1) * F],
                        start=(dp == 0),
                        stop=(dp == DC - 1),
                    )
                nc.vector.tensor_copy(qnat[:, nd * F:(nd + 1) * F], ps)
            stq = stp.tile([P, DC, P], f32r, name="stq", tag="st")
            for e in range(DC):
                pt = pst.tile([P, P], f32, name="pt_q", tag="pst")
                nc.tensor.transpose(pt, qnat[:, e * P:(e + 1) * P], ident)
                nc.vector.tensor_scalar_add(stq[:, e, :], pt, bq_sb[:, e:e + 1])
            nc.sync.dma_start(out=qtr[:, :, it * P:(it + 1) * P], in_=stq)

        # ---- Phase 2: per 512-query block: scores^T, exp, rowsum, PV ----
        vr = v_int.rearrange("(t p) d -> p t d", p=P)
        for t in range(NB):
            qb = qsp.tile([P, DC, F], f32r, name="qb", tag="qs")
            for c in range(DC):
                nc.sync.dma_start(out=qb[:, c, :], in_=qtr[:, c, t * F:(t + 1) * F])
            ex = big.tile([P, JT, F], f32r, name="ex", tag="big")
            rp = psr.tile([1, F], f32, name="rp", tag="psr")
            for jt in range(JT):
                ps = psm.tile([P, F], f32, name="ps_s", tag="psm")
                for e in range(DC):
                    nc.tensor.matmul(
                        ps,
                        lhsT=kt_sb[:, e, jt * P:(jt + 1) * P],
                        rhs=qb[:, e, :],
                        start=(e == 0),
                        stop=(e == DC - 1),
                    )
                nc.scalar.activation(ex[:, jt, :], ps, EXP)
                nc.tensor.matmul(
                    rp,
                    lhsT=ones,
                    rhs=ex[:, jt, :],
                    start=(jt == 0),
                    stop=(jt == JT - 1),
                    skip_group_check=True,
                )
            rs = stp.tile([1, F], f32, name="rs", tag="strs")
            nc.vector.tensor_copy(rs, rp)
            nc.sync.dma_start(out=rowsum[t:t + 1, :], in_=rs)
            for d in range(DC):
                vd = vip.tile([P, JT, P], f32r, name="vd", tag="vi")
                nc.sync.dma_start(out=vd[:, :8, :], in_=vr[:, :8, d * P:(d + 1) * P])
                nc.sync.dma_start(out=vd[:, 8:, :], in_=vr[:, 8:, d * P:(d + 1) * P])
                pv = psa.tile([P, F], f32, name="pv", tag="psa")
                for jt in range(JT):
                    nc.tensor.matmul(
                        pv,
                        lhsT=vd[:, jt, :],
                        rhs=ex[:, jt, :],
                        start=(jt == 0),
                        stop=(jt == JT - 1),
                    )
                ot = stp.tile([P, F], f32, name="ot", tag="st")
                nc.vector.tensor_copy(ot, pv)
                nc.sync.dma_start(
                    out=acct[d * P:(d + 1) * P, t * F:(t + 1) * F], in_=ot
                )

    nc.compile()
    return nc


def get_nc():
    if "nc" not in _CACHE:
        _CACHE["nc"] = _build_nc()
    return _CACHE["nc"]


def make_in_maps(query, key, value, Wq, bq, Wk, bk, Wv, bv):
    query = np.asarray(query, dtype=np.float32)
    key = np.asarray(key, dtype=np.float32)
    value = np.asarray(value, dtype=np.float32)
    wqt = np.ascontiguousarray(np.asarray(Wq, dtype=np.float32).T)
    wkt = np.ascontiguousarray(np.asarray(Wk, dtype=np.float32).T)
    wvt = np.ascontiguousarray(np.asarray(Wv, dtype=np.float32).T)
    bqt = np.ascontiguousarray(np.asarray(bq, dtype=np.float32).reshape(DC, P).T)
    in_maps = []
    for b in range(B):
        in_maps.append(
            {
                "xqt": np.ascontiguousarray(query[b].T),
                "xkt": np.ascontiguousarray(key[b].T),
                "xvt": np.ascontiguousarray(value[b].T),
                "wqt": wqt,
                "wkt": wkt,
                "wvt": wvt,
                "bqt": bqt,
            }
        )
    return in_maps


def postprocess(results, bv):
    bv = np.asarray(bv, dtype=np.float32)
    outs = []
    for b in range(B):
        acct = results[b]["acct"]              # [D, N] unnormalized (attn@v)^T
        rsum = results[b]["rowsum"].reshape(N)  # [N] softmax denominators
        outs.append(acct.T / rsum[:, None] + bv[None, :])
    return np.stack(outs).astype(np.float32)


def kernel(query, key, value, Wq, bq, Wk, bk, Wv, bv):
    from concourse.bass_utils import run_bass_kernel_spmd

    nc = get_nc()
    in_maps = make_in_maps(query, key, value, Wq, bq, Wk, bk, Wv, bv)
    res = run_bass_kernel_spmd(nc, in_maps, list(range(B)))
    return postprocess(res.results, bv)



# revision 12
# speedup vs baseline: 1.0933x; 1.0933x over previous
"""Bass/Trainium2 kernel for nn_CrossAttentionLayer.

out = softmax((x_q Wq^T + bq)(x_k Wk^T + bk)^T) (x_v Wv^T + bv)

Sharding: data-parallel over batch B=8 across the 8 NeuronCores.

Exact math simplifications used:
  - bk drops out of softmax (adds a per-row constant to the logits).
  - bv is added on the host (softmax rows sum to 1, so attn @ (v0 + bv)
    = attn @ v0 + bv).
  - softmax normalization (divide by row-sum) commutes with the PV
    matmul, so the device returns the unnormalized PV product plus
    per-partition partial row-sums; the host finishes the reduction
    and divides.
  - KEY ALGEBRAIC REDUCTION: the logits are
        scores = (Xq Wq^T + bq)(Xk Wk^T)^T
               = Xq (Wq^T Wk) Xk^T + (bq @ Wk) Xk^T.
    The host precomputes M = Wq^T @ Wk and c = bq @ Wk, so the device
    computes q'' = Xq M + c (ONE projection instead of two) and
    scores = q'' Xk^T against the RAW keys. This removes the whole
    k-projection (1/6 of total FLOPs) and every PE transpose (the
    projections are emitted directly in transposed layout because the
    contraction runs over the model dim, whose transposed input layout
    the host already provides).

Scheduling notes (cost-model-driven):
  - The DMA fabric is a single serial ~360 GB/s resource and dma_start
    calls FIFO through the issuing engine's sequencer. All INPUT DMAs
    are issued from nc.sync (SP) in consumption order; OUTPUT DMAs are
    issued from the engine that produces their data (Act/GpSimd), so an
    output waiting on compute can never block an input transfer.
  - q'' stays SBUF-RESIDENT, overlaid on wv's buffer (same pool tag;
    wv is dead once phase B ends), so there is no q'' DRAM round-trip
    at all. v (bf16) and exp(scores) (bf16) are also SBUF-resident;
    phase 2's only input traffic is the streamed raw keys.
  - PV PSUM->SBUF copies run on Act (not DVE): the per-block DVE
    row-sum reduce is ~8.6us and would otherwise stall the PV PSUM
    pool rotation (observed to drop the PE to its low p-state).
  - Weight loads stream as full [P, chunk, D] chunks interleaved with
    the xv tiles so the PE starts after ~3us and ramps while the DMA
    fabric fills the weights.
"""

import sys

if "/opt/trn_rl_repo" not in sys.path:
    sys.path.insert(0, "/opt/trn_rl_repo")

import numpy as np

B = 8          # batch == number of cores
D = 1024       # model/latent dim
N = 2048       # tokens (queries == keys)
P = 128        # partitions
DC = D // P    # 8 chunks of the d axis
KT = N // P    # 16 key tiles
F = 512        # matmul moving free dim (f32 PSUM max)
NB = N // F    # 4 query blocks

_CACHE = {}


def _build_nc():
    import concourse.bass as bass
    import concourse.mybir as mybir
    import concourse.tile as tile
    from concourse import bacc
    from contextlib import ExitStack

    f32 = mybir.dt.float32
    f32r = mybir.dt.float32r
    bf16 = mybir.dt.bfloat16
    EXP = mybir.ActivationFunctionType.Exp
    X = mybir.AxisListType.X
    ADD = mybir.AluOpType.add

    nc = bacc.Bacc("TRN2", target_bir_lowering=False, debug=False, num_devices=B)

    xqt = nc.dram_tensor("xqt", [D, N], f32r, kind="ExternalInput").ap()
    xkt = nc.dram_tensor("xkt", [D, N], f32r, kind="ExternalInput").ap()
    xvt = nc.dram_tensor("xvt", [D, N], f32r, kind="ExternalInput").ap()
    mqt = nc.dram_tensor("mqt", [D, D], f32r, kind="ExternalInput").ap()
    wvt = nc.dram_tensor("wvt", [D, D], f32r, kind="ExternalInput").ap()
    cqt = nc.dram_tensor("cqt", [P, DC], f32, kind="ExternalInput").ap()

    acct = nc.dram_tensor("acct", [D, N], f32, kind="ExternalOutput").ap()
    rsump = nc.dram_tensor("rsump", [NB, P, F], f32, kind="ExternalOutput").ap()

    with ExitStack() as ctx:
        tc = ctx.enter_context(tile.TileContext(nc))
        # SBUF pools; per-partition KB in comments, ~202 KB total.
        ovl = ctx.enter_context(tc.tile_pool(name="ovl", bufs=1))   # 64 (wv -> q'')
        mqp = ctx.enter_context(tc.tile_pool(name="mqp", bufs=1))   # 32
        vrp = ctx.enter_context(tc.tile_pool(name="vrp", bufs=1))   # 32
        xqp = ctx.enter_context(tc.tile_pool(name="xqp", bufs=2))   # 32
        xvp = ctx.enter_context(tc.tile_pool(name="xvp", bufs=3))   # 12
        xks = ctx.enter_context(tc.tile_pool(name="xks", bufs=3))   # 12
        exq = ctx.enter_context(tc.tile_pool(name="exq", bufs=1))   # 16
        otp = ctx.enter_context(tc.tile_pool(name="otp", bufs=2))   # 4
        rsp = ctx.enter_context(tc.tile_pool(name="rsp", bufs=1))   # 2
        cst = ctx.enter_context(tc.tile_pool(name="cst", bufs=1))
        psP = ctx.enter_context(tc.tile_pool(name="psP", bufs=2, space="PSUM"))
        psS = ctx.enter_context(tc.tile_pool(name="psS", bufs=2, space="PSUM"))
        psV = ctx.enter_context(tc.tile_pool(name="psV", bufs=4, space="PSUM"))

        wvr = wvt.rearrange("(c p) e -> p c e", p=P)
        mqr = mqt.rearrange("(c p) e -> p c e", p=P)
        xvr = xvt.rearrange("(c p) n -> p c n", p=P)
        xqr = xqt.rearrange("(c p) n -> p c n", p=P)
        xkr = xkt.rearrange("(c p) n -> p c n", p=P)

        # wv and q'' share one 64KB/partition buffer: q'''s writes pick
        # up a WAR dependency on phase B's last wv read, which is
        # naturally satisfied by program order on the PE.
        wv_sb = ovl.tile([P, DC, D], f32r, name="wv_sb", tag="ovl")
        mq_sb = mqp.tile([P, DC, D], f32r, name="mq_sb", tag="mq")
        c_sb = cst.tile([P, DC], f32, name="c_sb", tag="c_cq")
        v_sb = vrp.tile([P, KT, D], bf16, name="v_sb", tag="v")

        xv_tiles = {}

        def dma_xv(kt):
            xt = xvp.tile([P, DC, P], f32r, name="xt_v", tag="xv")
            nc.sync.dma_start(out=xt, in_=xvr[:, :, kt * P:(kt + 1) * P])
            xv_tiles[kt] = xt

        xq_tiles = {}

        def dma_xq(t):
            # Chunked so the serial DMA fabric never blocks a small
            # latency-critical transfer behind a 5.8us monolith.
            xt = xqp.tile([P, DC, F], f32r, name="xt_q", tag="xq")
            for c2 in range(0, DC, 2):
                nc.sync.dma_start(
                    out=xt[:, c2:c2 + 2, :],
                    in_=xqr[:, c2:c2 + 2, t * F:(t + 1) * F],
                )
            xq_tiles[t] = xt

        # --- input DMA prologue, in consumption order (SP FIFO) ---
        # xv0 split per chunk-pair and interleaved with the wv chunks:
        # the PE's first accumulation group starts after ~2us and
        # trickles at DMA pace while the weights land.
        xt0 = xvp.tile([P, DC, P], f32r, name="xt_v", tag="xv")
        for c2 in range(0, DC, 2):
            nc.sync.dma_start(
                out=xt0[:, c2:c2 + 2, :], in_=xvr[:, c2:c2 + 2, 0:P]
            )
            nc.sync.dma_start(out=wv_sb[:, c2 // 2, :], in_=wvr[:, c2 // 2, :])
        xv_tiles[0] = xt0
        dma_xv(1)
        for c in range(DC // 2, DC):
            nc.sync.dma_start(out=wv_sb[:, c, :], in_=wvr[:, c, :])
        dma_xv(2)

        # ---- Phase B: v = x_v Wv^T (no bias) -> SBUF-resident bf16 ----
        for kt in range(KT):
            # stay ~2 tiles ahead; slip one mq chunk per iteration and
            # xq(0) into the stream where they're harmless.
            nxt = kt + 3
            if nxt < KT:
                dma_xv(nxt)
            if 1 <= kt <= DC:
                nc.sync.dma_start(out=mq_sb[:, kt - 1, :], in_=mqr[:, kt - 1, :])
            elif kt == DC + 1:
                nc.sync.dma_start(out=c_sb, in_=cqt)
            elif kt == DC + 2:
                dma_xq(0)
            xt = xv_tiles.pop(kt)
            for eb in range(2):
                ps = psP.tile([P, F], f32, name="ps_v", tag="psP")
                for c in range(DC):
                    nc.tensor.matmul(
                        ps,
                        lhsT=xt[:, c, :],
                        rhs=wv_sb[:, c, eb * F:(eb + 1) * F],
                        start=(c == 0),
                        stop=(c == DC - 1),
                    )
                nc.scalar.copy(v_sb[:, kt, eb * F:(eb + 1) * F], ps)

        # ---- Phase A: q''T = (Xq M + c)^T -> SBUF (overlays wv) ----
        q_sb = ovl.tile([P, DC, N], f32r, name="q_sb", tag="ovl")
        for t in range(NB):
            if t + 1 < NB:
                dma_xq(t + 1)
            else:
                for kt in range(3):  # phase-2 key prefetch
                    dma_xk_head = xks.tile([P, DC, P], f32r, name="xk", tag="xk")
                    nc.sync.dma_start(
                        out=dma_xk_head, in_=xkr[:, :, kt * P:(kt + 1) * P]
                    )
                    xv_tiles[("xk", 0, kt)] = dma_xk_head
            xt = xq_tiles.pop(t)
            for e in range(DC):
                ps = psP.tile([P, F], f32, name="ps_q", tag="psP")
                for c in range(DC):
                    nc.tensor.matmul(
                        ps,
                        lhsT=mq_sb[:, c, e * P:(e + 1) * P],
                        rhs=xt[:, c, :],
                        start=(c == 0),
                        stop=(c == DC - 1),
                    )
                nc.vector.tensor_scalar_add(
                    q_sb[:, e, t * F:(t + 1) * F], ps, c_sb[:, e:e + 1]
                )

        # ---- Phase 2: per 512-query block: scores^T, exp, rowsum, PV ----
        xk_tiles = {
            (0, kt): xv_tiles.pop(("xk", 0, kt)) for kt in range(3)
        }

        def dma_xk(t, kt):
            xk = xks.tile([P, DC, P], f32r, name="xk", tag="xk")
            nc.sync.dma_start(out=xk, in_=xkr[:, :, kt * P:(kt + 1) * P])
            xk_tiles[(t, kt)] = xk

        for t in range(NB):
            ex = exq.tile([P, KT, F], bf16, name="ex", tag="ex")
            for kt in range(KT):
                if kt + 3 < KT:
                    dma_xk(t, kt + 3)
                elif t + 1 < NB:
                    dma_xk(t + 1, kt + 3 - KT)
                xk = xk_tiles.pop((t, kt))
                ps = psS.tile([P, F], f32, name="ps_s", tag="psS")
                for c in range(DC):
                    nc.tensor.matmul(
                        ps,
                        lhsT=xk[:, c, :],
                        rhs=q_sb[:, c, t * F:(t + 1) * F],
                        start=(c == 0),
                        stop=(c == DC - 1),
                    )
                nc.scalar.activation(ex[:, kt, :], ps, EXP)
            rs = rsp.tile([P, F], f32, name="rs", tag="rs")
            nc.vector.tensor_reduce(
                rs, ex.rearrange("p k f -> p f k"), axis=X, op=ADD
            )
            nc.gpsimd.dma_start(out=rsump[t], in_=rs)
            for d in range(DC):
                pv = psV.tile([P, F], f32, name="pv", tag="psV")
                for kt in range(KT):
                    nc.tensor.matmul(
                        pv,
                        lhsT=v_sb[:, kt, d * P:(d + 1) * P],
                        rhs=ex[:, kt, :],
                        start=(kt == 0),
                        stop=(kt == KT - 1),
                    )
                ot = otp.tile([P, F], f32, name="ot", tag="ot")
                nc.scalar.copy(ot, pv)
                nc.scalar.dma_start(
                    out=acct[d * P:(d + 1) * P, t * F:(t + 1) * F], in_=ot
                )

    nc.compile()
    return nc


def get_nc():
    if "nc" not in _CACHE:
        _CACHE["nc"] = _build_nc()
    return _CACHE["nc"]


def make_in_maps(query, key, value, Wq, bq, Wk, bk, Wv, bv):
    query = np.asarray(query, dtype=np.float32)
    key = np.asarray(key, dtype=np.float32)
    value = np.asarray(value, dtype=np.float32)
    Wq = np.asarray(Wq, dtype=np.float32)
    Wk = np.asarray(Wk, dtype=np.float32)
    Wv = np.asarray(Wv, dtype=np.float32)
    bq = np.asarray(bq, dtype=np.float32)
    # Host-side algebra (exact): M = Wq^T Wk, c = bq Wk
    mqt = np.ascontiguousarray(Wq.T @ Wk)
    cq = bq @ Wk
    cqt = np.ascontiguousarray(cq.reshape(DC, P).T)
    wvt = np.ascontiguousarray(Wv.T)
    in_maps = []
    for b in range(B):
        in_maps.append(
            {
                "xqt": np.ascontiguousarray(query[b].T),
                "xkt": np.ascontiguousarray(key[b].T),
                "xvt": np.ascontiguousarray(value[b].T),
                "mqt": mqt,
                "wvt": wvt,
                "cqt": cqt,
            }
        )
    return in_maps


def postprocess(results, bv):
    bv = np.asarray(bv, dtype=np.float32)
    outs = []
    for b in range(B):
        acct = results[b]["acct"]                # [D, N] unnormalized (attn@v)^T
        rsum = results[b]["rsump"].sum(axis=1).reshape(N)  # softmax denominators
        outs.append(acct.T / rsum[:, None] + bv[None, :])
    return np.stack(outs).astype(np.float32)


def kernel(query, key, value, Wq, bq, Wk, bk, Wv, bv):
    from concourse.bass_utils import run_bass_kernel_spmd

    nc = get_nc()
    in_maps = make_in_maps(query, key, value, Wq, bq, Wk, bk, Wv, bv)
    res = run_bass_kernel_spmd(nc, in_maps, list(range(B)))
    return postprocess(res.results, bv)


# revision 22
# speedup vs baseline: 30.4314x; 27.8333x over previous
"""Bass/Trainium2 kernel for nn_CrossAttentionLayer.

out = softmax((x_q Wq^T + bq)(x_k Wk^T + bk)^T) (x_v Wv^T + bv)

Sharding: data-parallel over batch B=8 across the 8 NeuronCores.

Exact math simplifications used:
  - bk drops out of softmax (adds a per-row constant to the logits).
  - bv is added on the host (softmax rows sum to 1, so attn @ (v0 + bv)
    = attn @ v0 + bv).
  - softmax normalization (divide by row-sum) commutes with the PV
    matmul, so the device returns the unnormalized PV product plus
    per-partition partial row-sums; the host finishes the reduction
    and divides.
  - KEY ALGEBRAIC REDUCTION: the logits are
        scores = (Xq Wq^T + bq)(Xk Wk^T)^T
               = Xq (Wq^T Wk) Xk^T + (bq @ Wk) Xk^T.
    The host precomputes M = Wq^T @ Wk and c = bq @ Wk, so the device
    computes q'' = Xq M + c (ONE projection instead of two) and
    scores = q'' Xk^T against the RAW keys. This removes the whole
    k-projection (1/6 of total FLOPs) and every PE transpose (the
    projections are emitted directly in transposed layout because the
    contraction runs over the model dim, whose transposed input layout
    the host already provides).

Scheduling notes (cost-model-driven):
  - The DMA fabric is a single serial ~360 GB/s resource and dma_start
    calls FIFO through the issuing engine's sequencer. All INPUT DMAs
    are issued from nc.sync (SP) in consumption order; OUTPUT DMAs are
    issued from the engine that produces their data (Act/GpSimd), so an
    output waiting on compute can never block an input transfer.
  - q'' stays SBUF-RESIDENT, overlaid on wv's buffer (same pool tag;
    wv is dead once phase B ends), so there is no q'' DRAM round-trip
    at all. v (bf16) and exp(scores) (bf16) are also SBUF-resident;
    phase 2's only input traffic is the streamed raw keys.
  - PV PSUM->SBUF copies run on Act (not DVE): the per-block DVE
    row-sum reduce is ~8.6us and would otherwise stall the PV PSUM
    pool rotation (observed to drop the PE to its low p-state).
  - Weight loads stream as full [P, chunk, D] chunks interleaved with
    the xv tiles so the PE starts after ~3us and ramps while the DMA
    fabric fills the weights.
"""

import sys

if "/opt/trn_rl_repo" not in sys.path:
    sys.path.insert(0, "/opt/trn_rl_repo")

import numpy as np

B = 8          # batch == number of cores
D = 1024       # model/latent dim
N = 2048       # tokens (queries == keys)
P = 128        # partitions
DC = D // P    # 8 chunks of the d axis
KT = N // P    # 16 key tiles
F = 512        # matmul moving free dim (f32 PSUM max)
NB = N // F    # 4 query blocks

_CACHE = {}


def _build_nc():
    import concourse.bass as bass
    import concourse.mybir as mybir
    import concourse.tile as tile
    from concourse import bacc
    from contextlib import ExitStack

    f32 = mybir.dt.float32
    f32r = mybir.dt.float32r
    bf16 = mybir.dt.bfloat16
    EXP = mybir.ActivationFunctionType.Exp
    X = mybir.AxisListType.X
    ADD = mybir.AluOpType.add

    nc = bacc.Bacc("TRN2", target_bir_lowering=False, debug=False, num_devices=B)

    xqt = nc.dram_tensor("xqt", [D, N], f32r, kind="ExternalInput").ap()
    xkt = nc.dram_tensor("xkt", [D, N], f32r, kind="ExternalInput").ap()
    # v path in bf16 (tolerates ~1e-3 noise): halves its DMA traffic.
    xvt = nc.dram_tensor("xvt", [D, N], bf16, kind="ExternalInput").ap()
    mqt = nc.dram_tensor("mqt", [D, D], f32r, kind="ExternalInput").ap()
    wvt = nc.dram_tensor("wvt", [D, D], bf16, kind="ExternalInput").ap()
    cqt = nc.dram_tensor("cqt", [P, DC], f32, kind="ExternalInput").ap()

    acct = nc.dram_tensor("acct", [D, N], f32, kind="ExternalOutput").ap()
    rsump = nc.dram_tensor("rsump", [NB, P, F], f32, kind="ExternalOutput").ap()

    with ExitStack() as ctx:
        tc = ctx.enter_context(tile.TileContext(nc))
        # SBUF pools; per-partition KB in comments, ~202 KB total.
        ovl = ctx.enter_context(tc.tile_pool(name="ovl", bufs=1))   # 64 (wv -> q'')
        mqp = ctx.enter_context(tc.tile_pool(name="mqp", bufs=1))   # 32
        vrp = ctx.enter_context(tc.tile_pool(name="vrp", bufs=1))   # 32
        xqp = ctx.enter_context(tc.tile_pool(name="xqp", bufs=2))   # 32
        xvp = ctx.enter_context(tc.tile_pool(name="xvp", bufs=3))   # 12
        xks = ctx.enter_context(tc.tile_pool(name="xks", bufs=3))   # 12
        exq = ctx.enter_context(tc.tile_pool(name="exq", bufs=1))   # 16
        otp = ctx.enter_context(tc.tile_pool(name="otp", bufs=2))   # 4
        rsp = ctx.enter_context(tc.tile_pool(name="rsp", bufs=1))   # 2
        cst = ctx.enter_context(tc.tile_pool(name="cst", bufs=1))
        psP = ctx.enter_context(tc.tile_pool(name="psP", bufs=2, space="PSUM"))
        psS = ctx.enter_context(tc.tile_pool(name="psS", bufs=2, space="PSUM"))
        psV = ctx.enter_context(tc.tile_pool(name="psV", bufs=4, space="PSUM"))

        wvr = wvt.rearrange("(c p) e -> p c e", p=P)
        mqr = mqt.rearrange("(c p) e -> p c e", p=P)
        xvr = xvt.rearrange("(c p) n -> p c n", p=P)
        xqr = xqt.rearrange("(c p) n -> p c n", p=P)
        xkr = xkt.rearrange("(c p) n -> p c n", p=P)

        # wv and q'' share one 64KB/partition buffer: q'''s writes pick
        # up a WAR dependency on phase B's last wv read, which is
        # naturally satisfied by program order on the PE.
        wv_sb = ovl.tile([P, DC, D], bf16, name="wv_sb", tag="ovl")
        mq_sb = mqp.tile([P, DC, D], f32r, name="mq_sb", tag="mq")
        c_sb = cst.tile([P, DC], f32, name="c_sb", tag="c_cq")
        v_sb = vrp.tile([P, KT, D], bf16, name="v_sb", tag="v")

        xv_tiles = {}

        def dma_xv(kt2):
            # double-width (2 key tiles) so bf16 runs stay >= 512B
            xt = xvp.tile([P, DC, 2 * P], bf16, name="xt_v", tag="xv")
            nc.sync.dma_start(out=xt, in_=xvr[:, :, kt2 * 2 * P:(kt2 + 1) * 2 * P])
            xv_tiles[kt2] = xt

        xq_tiles = {}

        def dma_xq(t):
            # Chunked so the serial DMA fabric never blocks a small
            # latency-critical transfer behind a 5.8us monolith.
            xt = xqp.tile([P, DC, F], f32r, name="xt_q", tag="xq")
            for c2 in range(0, DC, 2):
                nc.sync.dma_start(
                    out=xt[:, c2:c2 + 2, :],
                    in_=xqr[:, c2:c2 + 2, t * F:(t + 1) * F],
                )
            xq_tiles[t] = xt

        # --- input DMA prologue, in consumption order (SP FIFO) ---
        # xv double-tile 0 first, then the (bf16) wv chunks: the PE's
        # first accumulation group starts after ~2.5us and trickles at
        # DMA pace while the weights land.
        xt0 = xvp.tile([P, DC, 2 * P], bf16, name="xt_v", tag="xv")
        for c2 in range(0, DC, 2):
            nc.sync.dma_start(out=xt0[:, c2:c2 + 2, :], in_=xvr[:, c2:c2 + 2, 0:2 * P])
            nc.sync.dma_start(out=wv_sb[:, c2 // 2, :], in_=wvr[:, c2 // 2, :])
        xv_tiles[0] = xt0
        for c in range(DC // 2, DC):
            nc.sync.dma_start(out=wv_sb[:, c, :], in_=wvr[:, c, :])
        dma_xv(1)

        # ---- Phase B: v = x_v Wv^T (no bias) -> SBUF-resident bf16 ----
        for kt in range(KT):
            kt2, j = divmod(kt, 2)
            if j == 0:
                nxt = kt2 + 2
                if nxt < KT // 2:
                    dma_xv(nxt)
            # slip one mq chunk per iteration and xq(0) into the stream
            # where they're harmless.
            if 1 <= kt <= DC:
                nc.sync.dma_start(out=mq_sb[:, kt - 1, :], in_=mqr[:, kt - 1, :])
            elif kt == DC + 1:
                nc.sync.dma_start(out=c_sb, in_=cqt)
            elif kt == DC + 2:
                dma_xq(0)
            xt = xv_tiles[kt2]
            if j == 1:
                xv_tiles.pop(kt2)
            for eb in range(2):
                ps = psP.tile([P, F], f32, name="ps_v", tag="psP")
                for c in range(DC):
                    nc.tensor.matmul(
                        ps,
                        lhsT=xt[:, c, j * P:(j + 1) * P],
                        rhs=wv_sb[:, c, eb * F:(eb + 1) * F],
                        start=(c == 0),
                        stop=(c == DC - 1),
                    )
                nc.scalar.copy(v_sb[:, kt, eb * F:(eb + 1) * F], ps)

        # ---- Phase A: q''T = (Xq M + c)^T -> SBUF (overlays wv) ----
        q_sb = ovl.tile([P, DC, N], f32r, name="q_sb", tag="ovl")
        for t in range(NB):
            if t + 1 < NB:
                dma_xq(t + 1)
            else:
                for kt in range(3):  # phase-2 key prefetch
                    dma_xk_head = xks.tile([P, DC, P], f32r, name="xk", tag="xk")
                    nc.sync.dma_start(
                        out=dma_xk_head, in_=xkr[:, :, kt * P:(kt + 1) * P]
                    )
                    xv_tiles[("xk", 0, kt)] = dma_xk_head
            xt = xq_tiles.pop(t)
            for e in range(DC):
                ps = psP.tile([P, F], f32, name="ps_q", tag="psP")
                for c in range(DC):
                    nc.tensor.matmul(
                        ps,
                        lhsT=mq_sb[:, c, e * P:(e + 1) * P],
                        rhs=xt[:, c, :],
                        start=(c == 0),
                        stop=(c == DC - 1),
                    )
                nc.vector.tensor_scalar_add(
                    q_sb[:, e, t * F:(t + 1) * F], ps, c_sb[:, e:e + 1]
                )

        # ---- Phase 2: per 512-query block: scores^T, exp, rowsum, PV ----
        xk_tiles = {
            (0, kt): xv_tiles.pop(("xk", 0, kt)) for kt in range(3)
        }

        def dma_xk(t, kt):
            xk = xks.tile([P, DC, P], f32r, name="xk", tag="xk")
            nc.sync.dma_start(out=xk, in_=xkr[:, :, kt * P:(kt + 1) * P])
            xk_tiles[(t, kt)] = xk

        for t in range(NB):
            ex = exq.tile([P, KT, F], bf16, name="ex", tag="ex")
            for kt in range(KT):
                if kt + 3 < KT:
                    dma_xk(t, kt + 3)
                elif t + 1 < NB:
                    dma_xk(t + 1, kt + 3 - KT)
                xk = xk_tiles.pop((t, kt))
                ps = psS.tile([P, F], f32, name="ps_s", tag="psS")
                for c in range(DC):
                    nc.tensor.matmul(
                        ps,
                        lhsT=xk[:, c, :],
                        rhs=q_sb[:, c, t * F:(t + 1) * F],
                        start=(c == 0),
                        stop=(c == DC - 1),
                    )
                nc.scalar.activation(ex[:, kt, :], ps, EXP)
            rs = rsp.tile([P, F], f32, name="rs", tag="rs")
            nc.vector.tensor_reduce(
                rs, ex.rearrange("p k f -> p f k"), axis=X, op=ADD
            )
            nc.gpsimd.dma_start(out=rsump[t], in_=rs)
            for d in range(DC):
                pv = psV.tile([P, F], f32, name="pv", tag="psV")
                for kt in range(KT):
                    nc.tensor.matmul(
                        pv,
                        lhsT=v_sb[:, kt, d * P:(d + 1) * P],
                        rhs=ex[:, kt, :],
                        start=(kt == 0),
                        stop=(kt == KT - 1),
                    )
                ot = otp.tile([P, F], f32, name="ot", tag="ot")
                nc.scalar.copy(ot, pv)
                nc.scalar.dma_start(
                    out=acct[d * P:(d + 1) * P, t * F:(t + 1) * F], in_=ot
                )

    nc.compile()
    return nc


def get_nc():
    if "nc" not in _CACHE:
        _CACHE["nc"] = _build_nc()
    return _CACHE["nc"]


def make_in_maps(query, key, value, Wq, bq, Wk, bk, Wv, bv):
    import concourse.mybir as mybir

    np_bf16 = mybir.dt.np(mybir.dt.bfloat16)
    query = np.asarray(query, dtype=np.float32)
    key = np.asarray(key, dtype=np.float32)
    value = np.asarray(value, dtype=np.float32)
    Wq = np.asarray(Wq, dtype=np.float32)
    Wk = np.asarray(Wk, dtype=np.float32)
    Wv = np.asarray(Wv, dtype=np.float32)
    bq = np.asarray(bq, dtype=np.float32)
    # Host-side algebra (exact): M = Wq^T Wk, c = bq Wk
    mqt = np.ascontiguousarray(Wq.T @ Wk)
    cq = bq @ Wk
    cqt = np.ascontiguousarray(cq.reshape(DC, P).T)
    wvt = np.ascontiguousarray(Wv.T.astype(np_bf16))
    in_maps = []
    for b in range(B):
        in_maps.append(
            {
                "xqt": np.ascontiguousarray(query[b].T),
                "xkt": np.ascontiguousarray(key[b].T),
                "xvt": np.ascontiguousarray(value[b].T.astype(np_bf16)),
                "mqt": mqt,
                "wvt": wvt,
                "cqt": cqt,
            }
        )
    return in_maps


def postprocess(results, bv):
    bv = np.asarray(bv, dtype=np.float32)
    outs = []
    for b in range(B):
        acct = results[b]["acct"]                # [D, N] unnormalized (attn@v)^T
        rsum = results[b]["rsump"].sum(axis=1).reshape(N)  # softmax denominators
        outs.append(acct.T / rsum[:, None] + bv[None, :])
    return np.stack(outs).astype(np.float32)


def kernel(query, key, value, Wq, bq, Wk, bk, Wv, bv):
    from concourse.bass_utils import run_bass_kernel_spmd

    nc = get_nc()
    in_maps = make_in_maps(query, key, value, Wq, bq, Wk, bk, Wv, bv)
    res = run_bass_kernel_spmd(nc, in_maps, list(range(B)))
    return postprocess(res.results, bv)


# revision 25
# speedup vs baseline: 30.4383x; 1.0002x over previous
"""Bass/Trainium2 kernel for nn_CrossAttentionLayer.

out = softmax((x_q Wq^T + bq)(x_k Wk^T + bk)^T) (x_v Wv^T + bv)

Sharding: data-parallel over batch B=8 across the 8 NeuronCores.

Exact math simplifications used:
  - bk drops out of softmax (adds a per-row constant to the logits).
  - bv is added on the host (softmax rows sum to 1, so attn @ (v0 + bv)
    = attn @ v0 + bv).
  - softmax normalization (divide by row-sum) commutes with the PV
    matmul, so the device returns the unnormalized PV product plus
    per-partition partial row-sums; the host finishes the reduction
    and divides.
  - KEY ALGEBRAIC REDUCTION: the logits are
        scores = (Xq Wq^T + bq)(Xk Wk^T)^T
               = Xq (Wq^T Wk) Xk^T + (bq @ Wk) Xk^T.
    The host precomputes M = Wq^T @ Wk and c = bq @ Wk, so the device
    computes q'' = Xq M + c (ONE projection instead of two) and
    scores = q'' Xk^T against the RAW keys. This removes the whole
    k-projection (1/6 of total FLOPs) and every PE transpose (the
    projections are emitted directly in transposed layout because the
    contraction runs over the model dim, whose transposed input layout
    the host already provides).

Scheduling notes (cost-model-driven):
  - The DMA fabric is a single serial ~360 GB/s resource and dma_start
    calls FIFO through the issuing engine's sequencer. All INPUT DMAs
    are issued from nc.sync (SP) in consumption order; OUTPUT DMAs are
    issued from the engine that produces their data (Act/GpSimd), so an
    output waiting on compute can never block an input transfer.
  - q'' stays SBUF-RESIDENT, overlaid on wv's buffer (same pool tag;
    wv is dead once phase B ends), so there is no q'' DRAM round-trip
    at all. v (bf16) and exp(scores) (bf16) are also SBUF-resident;
    phase 2's only input traffic is the streamed raw keys.
  - PV PSUM->SBUF copies run on Act (not DVE): the per-block DVE
    row-sum reduce is ~8.6us and would otherwise stall the PV PSUM
    pool rotation (observed to drop the PE to its low p-state).
  - Weight loads stream as full [P, chunk, D] chunks interleaved with
    the xv tiles so the PE starts after ~3us and ramps while the DMA
    fabric fills the weights.
"""

import sys

if "/opt/trn_rl_repo" not in sys.path:
    sys.path.insert(0, "/opt/trn_rl_repo")

import numpy as np

B = 8          # batch == number of cores
D = 1024       # model/latent dim
N = 2048       # tokens (queries == keys)
P = 128        # partitions
DC = D // P    # 8 chunks of the d axis
KT = N // P    # 16 key tiles
F = 512        # matmul moving free dim (f32 PSUM max)
NB = N // F    # 4 query blocks

_CACHE = {}


def _build_nc():
    import concourse.bass as bass
    import concourse.mybir as mybir
    import concourse.tile as tile
    from concourse import bacc
    from contextlib import ExitStack

    f32 = mybir.dt.float32
    f32r = mybir.dt.float32r
    bf16 = mybir.dt.bfloat16
    EXP = mybir.ActivationFunctionType.Exp
    X = mybir.AxisListType.X
    ADD = mybir.AluOpType.add

    nc = bacc.Bacc("TRN2", target_bir_lowering=False, debug=False, num_devices=B)

    xqt = nc.dram_tensor("xqt", [D, N], f32r, kind="ExternalInput").ap()
    xkt = nc.dram_tensor("xkt", [D, N], f32r, kind="ExternalInput").ap()
    # v path in bf16 (tolerates ~1e-3 noise): halves its DMA traffic.
    xvt = nc.dram_tensor("xvt", [D, N], bf16, kind="ExternalInput").ap()
    mqt = nc.dram_tensor("mqt", [D, D], f32r, kind="ExternalInput").ap()
    wvt = nc.dram_tensor("wvt", [D, D], bf16, kind="ExternalInput").ap()
    cqt = nc.dram_tensor("cqt", [P, DC], f32, kind="ExternalInput").ap()

    acct = nc.dram_tensor("acct", [D, N], f32, kind="ExternalOutput").ap()
    rsump = nc.dram_tensor("rsump", [NB, P, F], f32, kind="ExternalOutput").ap()

    with ExitStack() as ctx:
        tc = ctx.enter_context(tile.TileContext(nc))
        # SBUF pools; per-partition KB in comments, ~202 KB total.
        ovl = ctx.enter_context(tc.tile_pool(name="ovl", bufs=1))   # 64 (wv -> q'')
        mqp = ctx.enter_context(tc.tile_pool(name="mqp", bufs=1))   # 32
        vrp = ctx.enter_context(tc.tile_pool(name="vrp", bufs=1))   # 32
        xqp = ctx.enter_context(tc.tile_pool(name="xqp", bufs=2))   # 32
        xvp = ctx.enter_context(tc.tile_pool(name="xvp", bufs=3))   # 12
        xks = ctx.enter_context(tc.tile_pool(name="xks", bufs=3))   # 12
        exq = ctx.enter_context(tc.tile_pool(name="exq", bufs=1))   # 16
        otp = ctx.enter_context(tc.tile_pool(name="otp", bufs=2))   # 4
        rsp = ctx.enter_context(tc.tile_pool(name="rsp", bufs=1))   # 2
        cst = ctx.enter_context(tc.tile_pool(name="cst", bufs=1))
        psP = ctx.enter_context(tc.tile_pool(name="psP", bufs=2, space="PSUM"))
        psS = ctx.enter_context(tc.tile_pool(name="psS", bufs=3, space="PSUM"))
        psV = ctx.enter_context(tc.tile_pool(name="psV", bufs=3, space="PSUM"))

        wvr = wvt.rearrange("(c p) e -> p c e", p=P)
        mqr = mqt.rearrange("(c p) e -> p c e", p=P)
        xvr = xvt.rearrange("(c p) n -> p c n", p=P)
        xqr = xqt.rearrange("(c p) n -> p c n", p=P)
        xkr = xkt.rearrange("(c p) n -> p c n", p=P)

        # wv and q'' share one 64KB/partition buffer: q'''s writes pick
        # up a WAR dependency on phase B's last wv read, which is
        # naturally satisfied by program order on the PE.
        wv_sb = ovl.tile([P, DC, D], bf16, name="wv_sb", tag="ovl")
        mq_sb = mqp.tile([P, DC, D], f32r, name="mq_sb", tag="mq")
        c_sb = cst.tile([P, DC], f32, name="c_sb", tag="c_cq")
        v_sb = vrp.tile([P, KT, D], bf16, name="v_sb", tag="v")

        xv_tiles = {}

        def dma_xv(kt2):
            # double-width (2 key tiles) so bf16 runs stay >= 512B
            xt = xvp.tile([P, DC, 2 * P], bf16, name="xt_v", tag="xv")
            nc.sync.dma_start(out=xt, in_=xvr[:, :, kt2 * 2 * P:(kt2 + 1) * 2 * P])
            xv_tiles[kt2] = xt

        xq_tiles = {}

        def dma_xq(t):
            # Chunked so the serial DMA fabric never blocks a small
            # latency-critical transfer behind a 5.8us monolith.
            xt = xqp.tile([P, DC, F], f32r, name="xt_q", tag="xq")
            for c2 in range(0, DC, 2):
                nc.sync.dma_start(
                    out=xt[:, c2:c2 + 2, :],
                    in_=xqr[:, c2:c2 + 2, t * F:(t + 1) * F],
                )
            xq_tiles[t] = xt

        # --- input DMA prologue, in consumption order (SP FIFO) ---
        # xv double-tile 0 first, then the (bf16) wv chunks: the PE's
        # first accumulation group starts after ~2.5us and trickles at
        # DMA pace while the weights land.
        xt0 = xvp.tile([P, DC, 2 * P], bf16, name="xt_v", tag="xv")
        for c2 in range(0, DC, 2):
            nc.sync.dma_start(out=xt0[:, c2:c2 + 2, :], in_=xvr[:, c2:c2 + 2, 0:2 * P])
            nc.sync.dma_start(out=wv_sb[:, c2 // 2, :], in_=wvr[:, c2 // 2, :])
        xv_tiles[0] = xt0
        for c in range(DC // 2, DC):
            nc.sync.dma_start(out=wv_sb[:, c, :], in_=wvr[:, c, :])
        dma_xv(1)

        # ---- Phase B: v = x_v Wv^T (no bias) -> SBUF-resident bf16 ----
        for kt in range(KT):
            kt2, j = divmod(kt, 2)
            if j == 0:
                nxt = kt2 + 2
                if nxt < KT // 2:
                    dma_xv(nxt)
            # slip one mq chunk per iteration and xq(0) into the stream
            # where they're harmless.
            if 1 <= kt <= DC:
                nc.sync.dma_start(out=mq_sb[:, kt - 1, :], in_=mqr[:, kt - 1, :])
            elif kt == DC + 1:
                nc.sync.dma_start(out=c_sb, in_=cqt)
            elif kt == DC + 2:
                dma_xq(0)
            xt = xv_tiles[kt2]
            if j == 1:
                xv_tiles.pop(kt2)
            for eb in range(2):
                ps = psP.tile([P, F], f32, name="ps_v", tag="psP")
                for c in range(DC):
                    nc.tensor.matmul(
                        ps,
                        lhsT=xt[:, c, j * P:(j + 1) * P],
                        rhs=wv_sb[:, c, eb * F:(eb + 1) * F],
                        start=(c == 0),
                        stop=(c == DC - 1),
                    )
                nc.scalar.copy(v_sb[:, kt, eb * F:(eb + 1) * F], ps)

        # ---- Phase A: q''T = (Xq M + c)^T -> SBUF (overlays wv) ----
        q_sb = ovl.tile([P, DC, N], f32r, name="q_sb", tag="ovl")
        for t in range(NB):
            if t + 1 < NB:
                dma_xq(t + 1)
            else:
                for kt in range(3):  # phase-2 key prefetch
                    dma_xk_head = xks.tile([P, DC, P], f32r, name="xk", tag="xk")
                    nc.sync.dma_start(
                        out=dma_xk_head, in_=xkr[:, :, kt * P:(kt + 1) * P]
                    )
                    xv_tiles[("xk", 0, kt)] = dma_xk_head
            xt = xq_tiles.pop(t)
            for e in range(DC):
                ps = psP.tile([P, F], f32, name="ps_q", tag="psP")
                for c in range(DC):
                    nc.tensor.matmul(
                        ps,
                        lhsT=mq_sb[:, c, e * P:(e + 1) * P],
                        rhs=xt[:, c, :],
                        start=(c == 0),
                        stop=(c == DC - 1),
                    )
                nc.vector.tensor_scalar_add(
                    q_sb[:, e, t * F:(t + 1) * F], ps, c_sb[:, e:e + 1]
                )

        # ---- Phase 2: per 512-query block: scores^T, exp, rowsum, PV ----
        xk_tiles = {
            (0, kt): xv_tiles.pop(("xk", 0, kt)) for kt in range(3)
        }

        def dma_xk(t, kt):
            xk = xks.tile([P, DC, P], f32r, name="xk", tag="xk")
            nc.sync.dma_start(out=xk, in_=xkr[:, :, kt * P:(kt + 1) * P])
            xk_tiles[(t, kt)] = xk

        for t in range(NB):
            ex = exq.tile([P, KT, F], bf16, name="ex", tag="ex")
            for kt in range(KT):
                if kt + 3 < KT:
                    dma_xk(t, kt + 3)
                elif t + 1 < NB:
                    dma_xk(t + 1, kt + 3 - KT)
                xk = xk_tiles.pop((t, kt))
                ps = psS.tile([P, F], f32, name="ps_s", tag="psS")
                for c in range(DC):
                    nc.tensor.matmul(
                        ps,
                        lhsT=xk[:, c, :],
                        rhs=q_sb[:, c, t * F:(t + 1) * F],
                        start=(c == 0),
                        stop=(c == DC - 1),
                    )
                nc.scalar.activation(ex[:, kt, :], ps, EXP)
            rs = rsp.tile([P, F], f32, name="rs", tag="rs")
            nc.vector.tensor_reduce(
                rs, ex.rearrange("p k f -> p f k"), axis=X, op=ADD
            )
            nc.gpsimd.dma_start(out=rsump[t], in_=rs)
            for d in range(DC):
                last = t == NB - 1 and d == DC - 1
                if not last:
                    pv = psV.tile([P, F], f32, name="pv", tag="psV")
                    for kt in range(KT):
                        nc.tensor.matmul(
                            pv,
                            lhsT=v_sb[:, kt, d * P:(d + 1) * P],
                            rhs=ex[:, kt, :],
                            start=(kt == 0),
                            stop=(kt == KT - 1),
                        )
                    ot = otp.tile([P, F], f32, name="ot", tag="ot")
                    nc.scalar.copy(ot, pv)
                    nc.scalar.dma_start(
                        out=acct[d * P:(d + 1) * P, t * F:(t + 1) * F], in_=ot
                    )
                else:
                    # very last output: two half-width PSUM groups so the
                    # first half's copy+DMA overlaps the second half's
                    # matmuls, shortening the end-of-kernel drain chain
                    for h in range(2):
                        hl, hh = h * (F // 2), (h + 1) * (F // 2)
                        pv = psV.tile([P, F // 2], f32, name="pv2", tag="psV")
                        for kt in range(KT):
                            nc.tensor.matmul(
                                pv,
                                lhsT=v_sb[:, kt, d * P:(d + 1) * P],
                                rhs=ex[:, kt, hl:hh],
                                start=(kt == 0),
                                stop=(kt == KT - 1),
                            )
                        ot = otp.tile([P, F // 2], f32, name="ot2", tag="ot")
                        nc.scalar.copy(ot, pv)
                        nc.scalar.dma_start(
                            out=acct[d * P:(d + 1) * P,
                                     t * F + hl:t * F + hh],
                            in_=ot,
                        )

    nc.compile()
    return nc


def get_nc():
    if "nc" not in _CACHE:
        _CACHE["nc"] = _build_nc()
    return _CACHE["nc"]


def make_in_maps(query, key, value, Wq, bq, Wk, bk, Wv, bv):
    import concourse.mybir as mybir

    np_bf16 = mybir.dt.np(mybir.dt.bfloat16)
    query = np.asarray(query, dtype=np.float32)
    key = np.asarray(key, dtype=np.float32)
    value = np.asarray(value, dtype=np.float32)
    Wq = np.asarray(Wq, dtype=np.float32)
    Wk = np.asarray(Wk, dtype=np.float32)
    Wv = np.asarray(Wv, dtype=np.float32)
    bq = np.asarray(bq, dtype=np.float32)
    # Host-side algebra (exact): M = Wq^T Wk, c = bq Wk
    mqt = np.ascontiguousarray(Wq.T @ Wk)
    cq = bq @ Wk
    cqt = np.ascontiguousarray(cq.reshape(DC, P).T)
    wvt = np.ascontiguousarray(Wv.T.astype(np_bf16))
    in_maps = []
    for b in range(B):
        in_maps.append(
            {
                "xqt": np.ascontiguousarray(query[b].T),
                "xkt": np.ascontiguousarray(key[b].T),
                "xvt": np.ascontiguousarray(value[b].T.astype(np_bf16)),
                "mqt": mqt,
                "wvt": wvt,
                "cqt": cqt,
            }
        )
    return in_maps


def postprocess(results, bv):
    bv = np.asarray(bv, dtype=np.float32)
    outs = []
    for b in range(B):
        acct = results[b]["acct"]                # [D, N] unnormalized (attn@v)^T
        rsum = results[b]["rsump"].sum(axis=1).reshape(N)  # softmax denominators
        outs.append(acct.T / rsum[:, None] + bv[None, :])
    return np.stack(outs).astype(np.float32)


def kernel(query, key, value, Wq, bq, Wk, bk, Wv, bv):
    from concourse.bass_utils import run_bass_kernel_spmd

    nc = get_nc()
    in_maps = make_in_maps(query, key, value, Wq, bq, Wk, bk, Wv, bv)
    res = run_bass_kernel_spmd(nc, in_maps, list(range(B)))
    return postprocess(res.results, bv)


# revision 26
# speedup vs baseline: 30.4417x; 1.0001x over previous
"""Bass/Trainium2 kernel for nn_CrossAttentionLayer.

out = softmax((x_q Wq^T + bq)(x_k Wk^T + bk)^T) (x_v Wv^T + bv)

Sharding: data-parallel over batch B=8 across the 8 NeuronCores.

Exact math simplifications used:
  - bk drops out of softmax (adds a per-row constant to the logits).
  - bv is added on the host (softmax rows sum to 1, so attn @ (v0 + bv)
    = attn @ v0 + bv).
  - softmax normalization (divide by row-sum) commutes with the PV
    matmul, so the device returns the unnormalized PV product plus
    per-partition partial row-sums; the host finishes the reduction
    and divides.
  - KEY ALGEBRAIC REDUCTION: the logits are
        scores = (Xq Wq^T + bq)(Xk Wk^T)^T
               = Xq (Wq^T Wk) Xk^T + (bq @ Wk) Xk^T.
    The host precomputes M = Wq^T @ Wk and c = bq @ Wk, so the device
    computes q'' = Xq M + c (ONE projection instead of two) and
    scores = q'' Xk^T against the RAW keys. This removes the whole
    k-projection (1/6 of total FLOPs) and every PE transpose (the
    projections are emitted directly in transposed layout because the
    contraction runs over the model dim, whose transposed input layout
    the host already provides).

Scheduling notes (cost-model-driven):
  - The DMA fabric is a single serial ~360 GB/s resource and dma_start
    calls FIFO through the issuing engine's sequencer. All INPUT DMAs
    are issued from nc.sync (SP) in consumption order; OUTPUT DMAs are
    issued from the engine that produces their data (Act/GpSimd), so an
    output waiting on compute can never block an input transfer.
  - q'' stays SBUF-RESIDENT, overlaid on wv's buffer (same pool tag;
    wv is dead once phase B ends), so there is no q'' DRAM round-trip
    at all. v (bf16) and exp(scores) (bf16) are also SBUF-resident;
    phase 2's only input traffic is the streamed raw keys.
  - PV PSUM->SBUF copies run on Act (not DVE): the per-block DVE
    row-sum reduce is ~8.6us and would otherwise stall the PV PSUM
    pool rotation (observed to drop the PE to its low p-state).
  - Weight loads stream as full [P, chunk, D] chunks interleaved with
    the xv tiles so the PE starts after ~3us and ramps while the DMA
    fabric fills the weights.
"""

import sys

if "/opt/trn_rl_repo" not in sys.path:
    sys.path.insert(0, "/opt/trn_rl_repo")

import numpy as np

B = 8          # batch == number of cores
D = 1024       # model/latent dim
N = 2048       # tokens (queries == keys)
P = 128        # partitions
DC = D // P    # 8 chunks of the d axis
KT = N // P    # 16 key tiles
F = 512        # matmul moving free dim (f32 PSUM max)
NB = N // F    # 4 query blocks

_CACHE = {}


def _build_nc():
    import concourse.bass as bass
    import concourse.mybir as mybir
    import concourse.tile as tile
    from concourse import bacc
    from contextlib import ExitStack

    f32 = mybir.dt.float32
    f32r = mybir.dt.float32r
    bf16 = mybir.dt.bfloat16
    EXP = mybir.ActivationFunctionType.Exp
    X = mybir.AxisListType.X
    ADD = mybir.AluOpType.add

    nc = bacc.Bacc("TRN2", target_bir_lowering=False, debug=False, num_devices=B)

    xqt = nc.dram_tensor("xqt", [D, N], f32r, kind="ExternalInput").ap()
    xkt = nc.dram_tensor("xkt", [D, N], f32r, kind="ExternalInput").ap()
    # v path in bf16 (tolerates ~1e-3 noise): halves its DMA traffic.
    xvt = nc.dram_tensor("xvt", [D, N], bf16, kind="ExternalInput").ap()
    mqt = nc.dram_tensor("mqt", [D, D], f32r, kind="ExternalInput").ap()
    wvt = nc.dram_tensor("wvt", [D, D], bf16, kind="ExternalInput").ap()
    cqt = nc.dram_tensor("cqt", [P, DC], f32, kind="ExternalInput").ap()

    acct = nc.dram_tensor("acct", [D, N], f32, kind="ExternalOutput").ap()
    rsump = nc.dram_tensor("rsump", [NB, P, F], f32, kind="ExternalOutput").ap()

    with ExitStack() as ctx:
        tc = ctx.enter_context(tile.TileContext(nc))
        # SBUF pools; per-partition KB in comments, ~202 KB total.
        ovl = ctx.enter_context(tc.tile_pool(name="ovl", bufs=1))   # 64 (wv -> q'')
        mqp = ctx.enter_context(tc.tile_pool(name="mqp", bufs=1))   # 32
        vrp = ctx.enter_context(tc.tile_pool(name="vrp", bufs=1))   # 32
        xqp = ctx.enter_context(tc.tile_pool(name="xqp", bufs=2))   # 32
        xvp = ctx.enter_context(tc.tile_pool(name="xvp", bufs=3))   # 12
        xks = ctx.enter_context(tc.tile_pool(name="xks", bufs=3))   # 12
        exq = ctx.enter_context(tc.tile_pool(name="exq", bufs=1))   # 16
        otp = ctx.enter_context(tc.tile_pool(name="otp", bufs=2))   # 4
        rsp = ctx.enter_context(tc.tile_pool(name="rsp", bufs=1))   # 2
        cst = ctx.enter_context(tc.tile_pool(name="cst", bufs=1))
        psP = ctx.enter_context(tc.tile_pool(name="psP", bufs=2, space="PSUM"))
        psS = ctx.enter_context(tc.tile_pool(name="psS", bufs=3, space="PSUM"))
        psV = ctx.enter_context(tc.tile_pool(name="psV", bufs=3, space="PSUM"))

        wvr = wvt.rearrange("(c p) e -> p c e", p=P)
        mqr = mqt.rearrange("(c p) e -> p c e", p=P)
        xvr = xvt.rearrange("(c p) n -> p c n", p=P)
        xqr = xqt.rearrange("(c p) n -> p c n", p=P)
        xkr = xkt.rearrange("(c p) n -> p c n", p=P)

        # wv and q'' share one 64KB/partition buffer: q'''s writes pick
        # up a WAR dependency on phase B's last wv read, which is
        # naturally satisfied by program order on the PE.
        wv_sb = ovl.tile([P, DC, D], bf16, name="wv_sb", tag="ovl")
        mq_sb = mqp.tile([P, DC, D], f32r, name="mq_sb", tag="mq")
        c_sb = cst.tile([P, DC], f32, name="c_sb", tag="c_cq")
        v_sb = vrp.tile([P, KT, D], bf16, name="v_sb", tag="v")

        xv_tiles = {}

        def dma_xv(kt2):
            # double-width (2 key tiles) so bf16 runs stay >= 512B
            xt = xvp.tile([P, DC, 2 * P], bf16, name="xt_v", tag="xv")
            nc.sync.dma_start(out=xt, in_=xvr[:, :, kt2 * 2 * P:(kt2 + 1) * 2 * P])
            xv_tiles[kt2] = xt

        xq_tiles = {}

        def dma_xq(t):
            # Chunked so the serial DMA fabric never blocks a small
            # latency-critical transfer behind a 5.8us monolith.
            xt = xqp.tile([P, DC, F], f32r, name="xt_q", tag="xq")
            for c2 in range(0, DC, 2):
                nc.sync.dma_start(
                    out=xt[:, c2:c2 + 2, :],
                    in_=xqr[:, c2:c2 + 2, t * F:(t + 1) * F],
                )
            xq_tiles[t] = xt

        # --- input DMA prologue, in consumption order (SP FIFO) ---
        # Phase B runs as two eb-passes so its first PSUM group is
        # enabled by only HALF of wv (1 MB): xv double-tile 0 pairs
        # interleave with the eb0 wv halves; eb1 halves, mq, xq(0)
        # stream in during pass 0.
        xt0 = xvp.tile([P, DC, 2 * P], bf16, name="xt_v", tag="xv")
        for c2 in range(0, DC, 2):
            nc.sync.dma_start(out=xt0[:, c2:c2 + 2, :], in_=xvr[:, c2:c2 + 2, 0:2 * P])
            nc.sync.dma_start(out=wv_sb[:, c2 // 2, 0:F], in_=wvr[:, c2 // 2, 0:F])
        xv_tiles[0] = xt0
        for c in range(DC // 2, DC):
            nc.sync.dma_start(out=wv_sb[:, c, 0:F], in_=wvr[:, c, 0:F])
        dma_xv(1)

        # ---- Phase B: v = x_v Wv^T (no bias) -> SBUF-resident bf16 ----
        # eb-pass structure; xv is re-streamed for pass 1 (the serial
        # DMA fabric has slack, SBUF does not).
        for pe in range(2):
            for kt in range(KT):
                kt2, j = divmod(kt, 2)
                s = pe * (KT // 2) + kt2
                if j == 0:
                    ns = s + 2
                    if ns < KT:
                        xtn = xvp.tile([P, DC, 2 * P], bf16, name="xt_v", tag="xv")
                        cc = (ns % (KT // 2)) * 2 * P
                        nc.sync.dma_start(out=xtn, in_=xvr[:, :, cc:cc + 2 * P])
                        xv_tiles[ns] = xtn
                if pe == 0:
                    # slip-ins: eb1 wv halves, c, xq(0) chunks, first mq
                    if 1 <= kt <= DC:
                        nc.sync.dma_start(
                            out=wv_sb[:, kt - 1, F:D], in_=wvr[:, kt - 1, F:D]
                        )
                    elif kt == DC + 1:
                        nc.sync.dma_start(out=c_sb, in_=cqt)
                    elif DC + 2 <= kt <= DC + 5:
                        c2 = (kt - DC - 2) * 2
                        if kt == DC + 2:
                            xq_tiles[0] = xqp.tile(
                                [P, DC, F], f32r, name="xt_q", tag="xq"
                            )
                        nc.sync.dma_start(
                            out=xq_tiles[0][:, c2:c2 + 2, :],
                            in_=xqr[:, c2:c2 + 2, 0:F],
                        )
                    elif kt >= DC + 6:
                        cm = kt - DC - 6
                        nc.sync.dma_start(out=mq_sb[:, cm, :], in_=mqr[:, cm, :])
                elif 0 <= kt < 6:
                    cm = kt + 2
                    nc.sync.dma_start(out=mq_sb[:, cm, :], in_=mqr[:, cm, :])
                xt = xv_tiles[s]
                if j == 1:
                    xv_tiles.pop(s)
                ps = psP.tile([P, F], f32, name="ps_v", tag="psP")
                for c in range(DC):
                    nc.tensor.matmul(
                        ps,
                        lhsT=xt[:, c, j * P:(j + 1) * P],
                        rhs=wv_sb[:, c, pe * F:(pe + 1) * F],
                        start=(c == 0),
                        stop=(c == DC - 1),
                    )
                nc.scalar.copy(v_sb[:, kt, pe * F:(pe + 1) * F], ps)

        # ---- Phase A: q''T = (Xq M + c)^T -> SBUF (overlays wv) ----
        q_sb = ovl.tile([P, DC, N], f32r, name="q_sb", tag="ovl")
        for t in range(NB):
            if t + 1 < NB:
                dma_xq(t + 1)  # t=0 tile was prefetched during phase B
            else:
                for kt in range(3):  # phase-2 key prefetch
                    dma_xk_head = xks.tile([P, DC, P], f32r, name="xk", tag="xk")
                    nc.sync.dma_start(
                        out=dma_xk_head, in_=xkr[:, :, kt * P:(kt + 1) * P]
                    )
                    xv_tiles[("xk", 0, kt)] = dma_xk_head
            xt = xq_tiles.pop(t)
            for e in range(DC):
                ps = psP.tile([P, F], f32, name="ps_q", tag="psP")
                for c in range(DC):
                    nc.tensor.matmul(
                        ps,
                        lhsT=mq_sb[:, c, e * P:(e + 1) * P],
                        rhs=xt[:, c, :],
                        start=(c == 0),
                        stop=(c == DC - 1),
                    )
                nc.vector.tensor_scalar_add(
                    q_sb[:, e, t * F:(t + 1) * F], ps, c_sb[:, e:e + 1]
                )

        # ---- Phase 2: per 512-query block: scores^T, exp, rowsum, PV ----
        xk_tiles = {
            (0, kt): xv_tiles.pop(("xk", 0, kt)) for kt in range(3)
        }

        def dma_xk(t, kt):
            xk = xks.tile([P, DC, P], f32r, name="xk", tag="xk")
            nc.sync.dma_start(out=xk, in_=xkr[:, :, kt * P:(kt + 1) * P])
            xk_tiles[(t, kt)] = xk

        for t in range(NB):
            ex = exq.tile([P, KT, F], bf16, name="ex", tag="ex")
            for kt in range(KT):
                if kt + 3 < KT:
                    dma_xk(t, kt + 3)
                elif t + 1 < NB:
                    dma_xk(t + 1, kt + 3 - KT)
                xk = xk_tiles.pop((t, kt))
                ps = psS.tile([P, F], f32, name="ps_s", tag="psS")
                for c in range(DC):
                    nc.tensor.matmul(
                        ps,
                        lhsT=xk[:, c, :],
                        rhs=q_sb[:, c, t * F:(t + 1) * F],
                        start=(c == 0),
                        stop=(c == DC - 1),
                    )
                nc.scalar.activation(ex[:, kt, :], ps, EXP)
            rs = rsp.tile([P, F], f32, name="rs", tag="rs")
            nc.vector.tensor_reduce(
                rs, ex.rearrange("p k f -> p f k"), axis=X, op=ADD
            )
            nc.gpsimd.dma_start(out=rsump[t], in_=rs)
            for d in range(DC):
                last = t == NB - 1 and d == DC - 1
                if not last:
                    pv = psV.tile([P, F], f32, name="pv", tag="psV")
                    for kt in range(KT):
                        nc.tensor.matmul(
                            pv,
                            lhsT=v_sb[:, kt, d * P:(d + 1) * P],
                            rhs=ex[:, kt, :],
                            start=(kt == 0),
                            stop=(kt == KT - 1),
                        )
                    ot = otp.tile([P, F], f32, name="ot", tag="ot")
                    nc.scalar.copy(ot, pv)
                    nc.scalar.dma_start(
                        out=acct[d * P:(d + 1) * P, t * F:(t + 1) * F], in_=ot
                    )
                else:
                    # very last output: two half-width PSUM groups so the
                    # first half's copy+DMA overlaps the second half's
                    # matmuls, shortening the end-of-kernel drain chain
                    for h in range(2):
                        hl, hh = h * (F // 2), (h + 1) * (F // 2)
                        pv = psV.tile([P, F // 2], f32, name="pv2", tag="psV")
                        for kt in range(KT):
                            nc.tensor.matmul(
                                pv,
                                lhsT=v_sb[:, kt, d * P:(d + 1) * P],
                                rhs=ex[:, kt, hl:hh],
                                start=(kt == 0),
                                stop=(kt == KT - 1),
                            )
                        ot = otp.tile([P, F // 2], f32, name="ot2", tag="ot")
                        nc.scalar.copy(ot, pv)
                        nc.scalar.dma_start(
                            out=acct[d * P:(d + 1) * P,
                                     t * F + hl:t * F + hh],
                            in_=ot,
                        )

    nc.compile()
    return nc


def get_nc():
    if "nc" not in _CACHE:
        _CACHE["nc"] = _build_nc()
    return _CACHE["nc"]


def make_in_maps(query, key, value, Wq, bq, Wk, bk, Wv, bv):
    import concourse.mybir as mybir

    np_bf16 = mybir.dt.np(mybir.dt.bfloat16)
    query = np.asarray(query, dtype=np.float32)
    key = np.asarray(key, dtype=np.float32)
    value = np.asarray(value, dtype=np.float32)
    Wq = np.asarray(Wq, dtype=np.float32)
    Wk = np.asarray(Wk, dtype=np.float32)
    Wv = np.asarray(Wv, dtype=np.float32)
    bq = np.asarray(bq, dtype=np.float32)
    # Host-side algebra (exact): M = Wq^T Wk, c = bq Wk
    mqt = np.ascontiguousarray(Wq.T @ Wk)
    cq = bq @ Wk
    cqt = np.ascontiguousarray(cq.reshape(DC, P).T)
    wvt = np.ascontiguousarray(Wv.T.astype(np_bf16))
    in_maps = []
    for b in range(B):
        in_maps.append(
            {
                "xqt": np.ascontiguousarray(query[b].T),
                "xkt": np.ascontiguousarray(key[b].T),
                "xvt": np.ascontiguousarray(value[b].T.astype(np_bf16)),
                "mqt": mqt,
                "wvt": wvt,
                "cqt": cqt,
            }
        )
    return in_maps


def postprocess(results, bv):
    bv = np.asarray(bv, dtype=np.float32)
    outs = []
    for b in range(B):
        acct = results[b]["acct"]                # [D, N] unnormalized (attn@v)^T
        rsum = results[b]["rsump"].sum(axis=1).reshape(N)  # softmax denominators
        outs.append(acct.T / rsum[:, None] + bv[None, :])
    return np.stack(outs).astype(np.float32)


def kernel(query, key, value, Wq, bq, Wk, bk, Wv, bv):
    from concourse.bass_utils import run_bass_kernel_spmd

    nc = get_nc()
    in_maps = make_in_maps(query, key, value, Wq, bq, Wk, bk, Wv, bv)
    res = run_bass_kernel_spmd(nc, in_maps, list(range(B)))
    return postprocess(res.results, bv)
